# revision 22
# baseline (speedup 1.0000x reference)
"""NewsEncoder (Fastformer) Trainium2 Bass kernel.

Contract: kernel(**inputs) takes FULL inputs (tokens [8192,64], emb_table
[50000,256], WQ/WK/WV/WO [256,256], dense_w [256,1], dense_b [1]) and
returns the FULL output news_vector [8192, 256] f32.

Strategy: pure data parallel over 8 NeuronCores (1024 seqs each). Per core,
32 chunks x 32 seqs (2048 tokens). Embedding rows are gathered via indirect
DMA (one [128]-row gather per token-tile, the only offset shape this walrus
unrolls correctly) with inline f32->bf16 cast.

Math restructure (avoids materializing Q/K/V/hidden per token):
  w_pre[t,h] = x[t] . z_{h,s}        z from per-seq means via Z = WK^T Gq + WQ^T Gk
  sv[t,h]    = x[t] . m_h            m_h = WV_h @ (WO @ dense_w)_h   (host)
  scores[t]  = sum_h softmax_w[t,h] * sv[t,h]  (wdinv folded into sc lhsT,
               score rows replicated to all 64 (h,s) rows via seld64)
  attn       = softmax_t(scores);  U[t,(h,s)] = attn[t] * w[t,h]
               (U = (wn*nrow)*em with nrow = wdinv*dinv: 4x ts + 2x tt)
  y_T        = x_tile^T @ U          (each 128-tok tile covers 2 seqs: only
               its 16 live (s,h) columns are produced, no accum chains)
  cpool      = WV_h^T-slices @ y_T;  nv = WO^T @ cpool
All matmuls bf16 with f32 PSUM accumulation.
"""

import sys

sys.path.insert(0, "/opt/trn_rl_repo")

import numpy as np
import ml_dtypes

import concourse.bass as bass
import concourse.tile as tile
from concourse import mybir
from concourse.bass_utils import run_bass_kernel_spmd
from concourse.tile import ScopedClock

BF16 = mybir.dt.bfloat16
F32 = mybir.dt.float32
I32 = mybir.dt.int32
NPBF = ml_dtypes.bfloat16

VOCAB, D = 50000, 256
B, L = 8192, 64
H, DH = 8, 32
# split embedding table for int16 SWDGE dma_gather: A = rows [0, 32767) plus a
# zero row at 32767; B = rows [32767, 50000) plus a zero row at the end.
# Out-of-half tokens index the zero row; merged with one Pool add.
NA = 32767
NB = VOCAB - NA                      # 17233
I16 = mybir.dt.int16
NCORES = 8
SEQ_PER_CORE = B // NCORES          # 1024
CHUNKS = 32                          # per core
SEQ_PER_CHUNK = SEQ_PER_CORE // CHUNKS   # 32
TOK_PER_CHUNK = SEQ_PER_CHUNK * L        # 2048
TILES_PER_CHUNK = TOK_PER_CHUNK // 128   # 16
SUBTILES = 4                         # per chunk; 512 tokens / 8 seqs each
T_SUB = 512


# ---------------------------------------------------------------------------
# Walrus on this toolchain encodes at most ONE sem-wait per Drain; split the
# TileContext tail-drain waits across a chain of drains.
def _patched_drain_and_barrier(self, tick_clock, wait_clock):
    d = self.nc.sync.drain()
    wait_clock.add_sem_waits(d.ins, ScopedClock({None: tick_clock.global_clock}))
    si = d.ins.sync_info
    if si is not None and si.on_wait and len(si.on_wait) > 1:
        waits = list(si.on_wait)
        si.on_wait = waits[:1]
        for w in waits[1:]:
            d2 = self.nc.sync.drain()
            si2 = d2.ins.sync_info
            if si2 is None:
                d2.ins.sync_info = mybir.SyncInfo(on_wait=[w], on_update=[])
            else:
                si2.on_wait = [w]
    self.nc.all_engine_barrier()
    assert self.sems is not None
    popped = self.nc._tile_sem_poison_stack.pop()
    assert popped is self._sem_poison
    sems = list(self.sems.allocated().values())
    for i in range(0, len(sems), 16):
        self.nc.clear_and_free_semaphores(sems[i:i + 16])
    self.nc.all_engine_barrier()


tile.TileContext._drain_and_barrier = _patched_drain_and_barrier

# Regular instructions are also limited in wait-slot count; split excess
# waits onto same-engine NoOps inserted just before the instruction.
MAX_WAITS = 1
_orig_lower_ordered = tile.TileContext._lower_ordered_insts


def _split_waits_lower(self, ordered):
    for bb_name, insts in ordered.items():
        out = []
        for inst in insts:
            si = getattr(inst, "sync_info", None)
            if si is not None and si.on_wait and len(si.on_wait) > MAX_WAITS:
                waits = list(si.on_wait)
                extra, keep = waits[:-MAX_WAITS], waits[-MAX_WAITS:]
                for i in range(0, len(extra), MAX_WAITS):
                    nop = mybir.InstNoOp(
                        name=f"WS-{self.nc.next_id()}",
                        sync_info=mybir.SyncInfo(
                            on_wait=extra[i:i + MAX_WAITS], on_update=[]),
                        bass_nofuse=True,
                        engine=inst.engine,
                    )
                    out.append(nop)
                si.on_wait = keep
            out.append(inst)
        insts[:] = out
    return _orig_lower_ordered(self, ordered)


tile.TileContext._lower_ordered_insts = _split_waits_lower


def _install_ntff_hook():
    """Register the axon NTFF profile hook if the image's antenv lacks it."""
    try:
        import antenv.axon_hooks  # noqa: F401
        return
    except ImportError:
        pass
    try:
        import types
        if "/root/.axon_site" not in sys.path:
            sys.path.insert(0, "/root/.axon_site")
        from trn_agent_boot.trn_boot import _ntff_profile_via_ctypes
        hook = _ntff_profile_via_ctypes("/opt/axon/libaxon_pjrt.so")
        import antenv
        mod = types.ModuleType("antenv.axon_hooks")
        mod.get_axon_ntff_profile_hook = lambda: hook
        mod.set_axon_ntff_profile_hook = lambda h: None
        sys.modules["antenv.axon_hooks"] = mod
        antenv.axon_hooks = mod
    except Exception:
        pass


_install_ntff_hook()


def _ap(t_ap: bass.AP, extra_offset: int, ap_list) -> bass.AP:
    """Manual AP on a tile's tensor with explicit [step, count] axes."""
    return bass.AP(tensor=t_ap.tensor, offset=t_ap.offset + extra_offset, ap=ap_list)


def build_nc(n_chunks: int = CHUNKS) -> bass.Bass:
    nc = bass.Bass("TRN2", target_bir_lowering=False, debug=False,
                   num_devices=NCORES)

    emb = nc.declare_dram_parameter("emb", [VOCAB, D], BF16, isOutput=False)
    idx_d = nc.declare_dram_parameter("idx", [128, n_chunks * TILES_PER_CHUNK],
                                      I32, isOutput=False)
    # packed bf16 constants, all [128, cols]
    wq_d = nc.declare_dram_parameter("wq", [128, 2 * 256], BF16, isOutput=False)
    wk_d = nc.declare_dram_parameter("wk", [128, 2 * 256], BF16, isOutput=False)
    wkt_d = nc.declare_dram_parameter("wkt", [128, 2 * 256], BF16, isOutput=False)
    wqt_d = nc.declare_dram_parameter("wqt", [128, 2 * 256], BF16, isOutput=False)
    wv_d = nc.declare_dram_parameter("wv", [128, 2 * 256], BF16, isOutput=False)
    wo_d = nc.declare_dram_parameter("wo", [128, 2 * 256], BF16, isOutput=False)
    wpsc_d = nc.declare_dram_parameter("wpsc", [128, 2 * 64], BF16, isOutput=False)
    mh_d = nc.declare_dram_parameter("maskhead", [128, 2 * 256], BF16,
                                     isOutput=False)
    ident_d = nc.declare_dram_parameter("ident", [128, 128], BF16, isOutput=False)
    m64_d = nc.declare_dram_parameter("mask64", [64, 512], BF16, isOutput=False)
    seld64_d = nc.declare_dram_parameter("seld64", [64, 64], BF16, isOutput=False)

    out_d = nc.declare_dram_parameter(
        "out", [n_chunks * SEQ_PER_CHUNK, D], F32, isOutput=True)

    with tile.TileContext(nc) as tc:
        _build_body(nc, tc, n_chunks, emb, idx_d, wq_d, wk_d, wkt_d,
                    wqt_d, wv_d, wo_d, wpsc_d, mh_d, ident_d, m64_d,
                    seld64_d, out_d)
    return nc


def _build_body(nc, tc, n_chunks, emb, idx_d, wq_d, wk_d, wkt_d, wqt_d,
                wv_d, wo_d, wpsc_d, mh_d, ident_d, m64_d, seld64_d, out_d):
    from contextlib import ExitStack
    ctx = ExitStack()
    with ctx:
        consts = ctx.enter_context(tc.tile_pool(name="consts", bufs=1))
        xpool = ctx.enter_context(tc.tile_pool(name="x", bufs=3))
        xtp = ctx.enter_context(tc.tile_pool(name="xt", bufs=3))
        sb2 = ctx.enter_context(tc.tile_pool(name="sb2", bufs=3))
        sb3 = ctx.enter_context(tc.tile_pool(name="sb3", bufs=8))
        outp = ctx.enter_context(tc.tile_pool(name="outp", bufs=3))
        pss = ctx.enter_context(tc.tile_pool(name="pss", bufs=1, space="PSUM"))
        psw = ctx.enter_context(tc.tile_pool(name="psw", bufs=2, space="PSUM"))
        pst = ctx.enter_context(tc.tile_pool(name="pst", bufs=1, space="PSUM"))
        psu = ctx.enter_context(tc.tile_pool(name="psu", bufs=1, space="PSUM"))
        pssc = ctx.enter_context(tc.tile_pool(name="pssc", bufs=1, space="PSUM"))
        psy = ctx.enter_context(tc.tile_pool(name="psy", bufs=1, space="PSUM"))
        ps2 = ctx.enter_context(tc.tile_pool(name="ps2", bufs=1, space="PSUM"))

        # ---- load constants ------------------------------------------------
        def cload(dram, shape, name):
            t = consts.tile(shape, BF16, tag=name)
            nc.sync.dma_start(out=t[:], in_=dram[:].rearrange(
                "p (a b) -> p a b", a=shape[1]) if len(shape) == 3 else dram[:])
            return t

        idx_sb = consts.tile([128, n_chunks * TILES_PER_CHUNK], I32, tag="idx")
        nc.sync.dma_start(out=idx_sb[:], in_=idx_d[:])
        wq = cload(wq_d, [128, 2, 256], "wq")
        wk = cload(wk_d, [128, 2, 256], "wk")
        wkt = cload(wkt_d, [128, 2, 256], "wkt")
        wqt = cload(wqt_d, [128, 2, 256], "wqt")
        wv = cload(wv_d, [128, 2, 256], "wv")
        wo = cload(wo_d, [128, 2, 256], "wo")
        mhd = cload(mh_d, [128, 2, 256], "maskhead")
        ident = cload(ident_d, [128, 128], "ident")
        m64 = cload(m64_d, [64, 512], "m64")
        seld64 = cload(seld64_d, [64, 64], "seld64")

        # wps lhsT tiles (double-buffered manually; const cols written once)
        wps = [consts.tile([128, 2, 4, 128], BF16, tag=f"wps{i}", name=f"wps{i}") for i in (0, 1)]
        wpsc = consts.tile([128, 2, 64], BF16, tag="wpsc")
        nc.sync.dma_start(out=wpsc[:], in_=wpsc_d[:].rearrange(
            "p (a b) -> p a b", a=2))
        for i in (0, 1):
            for k in (0, 1):
                for st in range(SUBTILES):
                    nc.vector.tensor_copy(out=wps[i][:, k, st, 64:128],
                                          in_=wpsc[:, k, :])

        Exp = mybir.ActivationFunctionType.Exp
        Copy = mybir.ActivationFunctionType.Copy
        Mult = mybir.AluOpType.mult
        Add = mybir.AluOpType.add

        # ---- chunk loop ----------------------------------------------------
        for c in range(n_chunks):
            wpsi = wps[c % 2]
            # gather: one indirect DMA per 128-token tile (the only offset
            # shape this walrus unrolls correctly), into one chunk tile.
            x = xpool.tile([128, TILES_PER_CHUNK, 256], BF16, tag="x")
            for j in range(TILES_PER_CHUNK):
                g = c * TILES_PER_CHUNK + j
                nc.gpsimd.indirect_dma_start(
                    out=x[:, j, :], out_offset=None, in_=emb[:],
                    in_offset=bass.IndirectOffsetOnAxis(
                        ap=idx_sb[:, g:g + 1], axis=0))

            # x_T via PE transposes: xT [128, 32, 128] bf16 where free idx
            # (b, t): b = 2j+k (tile j, d-half k), t = in-tile token.
            # Chunk-token = 128j + t.
            xT = xtp.tile([128, 2 * TILES_PER_CHUNK, 128], BF16, tag="xT")
            for q in range(4):
                tp = pst.tile([128, 2, 512], BF16, tag="xTps")
                for jj in range(4):
                    for k in (0, 1):
                        nc.tensor.transpose(
                            out=tp[:, k, 128 * jj:128 * jj + 128],
                            in_=x[:, 4 * q + jj, 128 * k:128 * k + 128],
                            identity=ident[:])
                oap0 = _ap(xT[:, 8 * q, :], 0,
                           [[xT[:].ap[0][0], 128], [256, 4], [1, 128]])
                nc.vector.tensor_copy(out=oap0, in_=tp[:, 0, :])
                oap1 = _ap(xT[:, 8 * q + 1, :], 0,
                           [[xT[:].ap[0][0], 128], [256, 4], [1, 128]])
                nc.scalar.activation(out=oap1, in_=tp[:, 1, :], func=Copy)

            def xT_k(k, st):
                # [128, 4, 128] AP over xT: d-half k, subtile st (512 tokens)
                return _ap(xT[:, 8 * st + k, :], 0,
                           [[xT[:].ap[0][0], 128], [256, 4], [1, 128]])

            # xbar (sum over L; 1/64 folded into wq/wk host-side): [128,2,32]
            xb = sb2.tile([128, 2, SEQ_PER_CHUNK], BF16, tag="xb")
            with nc.allow_low_precision(reason="xbar bf16 ok (means of 64)"):
                for k in (0, 1):
                    src = _ap(xT[:, k, :], 0,
                              [[xT[:].ap[0][0], 128], [256, 16], [64, 2],
                               [1, 64]])
                    nc.vector.reduce_sum(
                        out=xb[:, k, :].rearrange("p (a b) -> p a b", a=16),
                        in_=src, axis=mybir.AxisListType.X)

            # gq/gk: psum [128, 2(m), 64]  (cols 0:32 gq, 32:64 gk)
            gqp = pss.tile([128, 2, 64], F32, tag="seqstats")
            for m in (0, 1):
                for t, w_ in ((0, wq), (1, wk)):
                    for k in (0, 1):
                        nc.tensor.matmul(
                            out=gqp[:, m, 32 * t:32 * t + 32],
                            lhsT=w_[:, k, 128 * m:128 * m + 128],
                            rhs=xb[:, k, :],
                            start=(k == 0), stop=(k == 1))
            gqs = sb2.tile([128, 2, 64], BF16, tag="gqs")
            nc.scalar.activation(out=gqs[:], in_=gqp[:], func=Copy)

            # masked replicated means: Gm[t][kp] [128, 256] bf16
            Gm = [[sb2.tile([128, 256], BF16, tag=f"gm{t}{kp}", name=f"gm{t}{kp}")
                   for kp in (0, 1)] for t in (0, 1)]
            for t in (0, 1):
                for kp in (0, 1):
                    src = _ap(gqs[:, kp, 32 * t:32 * t + 32], 0,
                              [[gqs[:].ap[0][0], 128], [8, 4], [0, 8], [1, 8]])
                    nc.vector.tensor_tensor(
                        out=Gm[t][kp][:].rearrange(
                            "p (a b c) -> p a b c", a=4, b=8),
                        in0=src,
                        in1=mhd[:, kp, :].rearrange(
                            "p (a b c) -> p a b c", a=4, b=8),
                        op=Mult)

            # Z: psum [128, 2(m2), 256] f32, accumulate 4 (kp x term)
            zp = pss.tile([128, 2, 256], F32, tag="seqstats")
            for m2 in (0, 1):
                first = True
                for kp in (0, 1):
                    for t, wt_ in ((0, wkt), (1, wqt)):
                        nc.tensor.matmul(
                            out=zp[:, m2, :],
                            lhsT=wt_[:, kp, 128 * m2:128 * m2 + 128],
                            rhs=Gm[t][kp][:],
                            start=first, stop=(kp == 1 and t == 1))
                        first = False
            # one merged strided ACT copy for both m2 halves
            nc.scalar.activation(
                out=wpsi[:, :, :, 0:64],
                in_=zp[:].rearrange("p m (st c) -> p m st c", c=64),
                func=Copy)

            # yT columns laid out (seq-in-chunk, h); each (st, j2, m2) matmul
            # writes its contiguous 16-col block, no accumulation chains.
            yTp = psy.tile([128, 2, 32, 8], F32, tag="yT")

            for st in range(SUBTILES):
                # W = [w_pre | sv_rep] : psum [128, 512]
                Wp = psw.tile([128, 512], F32, tag="W")
                for k in (0, 1):
                    nc.tensor.matmul(
                        out=Wp[:], lhsT=wpsi[:, k, st, :],
                        rhs=xT_k(k, st),
                        start=(k == 0), stop=(k == 1))
                ew = sb3.tile([64, 512], BF16, tag="ew")
                nc.scalar.activation(out=ew[:], in_=Wp[0:64, :], func=Exp)
                svs = sb3.tile([64, 512], BF16, tag="svs")
                nc.scalar.activation(out=svs[:], in_=Wp[64:128, :], func=Copy)

                # fused: wn = ew*m64 and wden = row-sum(wn) in one DVE op
                wn = sb3.tile([64, 512], BF16, tag="wn")
                wden = sb3.tile([64, 1], F32, tag="wden")
                nc.vector.scalar_tensor_tensor(
                    out=wn[:], in0=ew[:], scalar=1.0, in1=m64[:],
                    op0=Mult, op1=Mult, accum_out=wden[:])
                wdinv = sb3.tile([64, 1], F32, tag="wdinv")
                nc.vector.reciprocal(out=wdinv[:], in_=wden[:])
                # wd64[hs, 8h'+s'] = wdinv[hs] * (s==s'): sc64 lhsT replicates
                # the scores to all 64 (h',s') rows, killing the EB broadcast
                wd64 = sb3.tile([64, 64], BF16, tag="wd64")
                nc.vector.tensor_scalar(
                    out=wd64[:], in0=seld64[:], scalar1=wdinv[:], scalar2=None,
                    op0=Mult)
                # P = wn * sv  (both SBUF bf16 -> 2x-mode tensor_tensor)
                P = sb3.tile([64, 512], BF16, tag="P")
                nc.vector.tensor_tensor(out=P[:], in0=wn[:], in1=svs[:],
                                        op=Mult)

                scp = pssc.tile([64, 512], F32, tag="sc")
                nc.tensor.matmul(out=scp[:], lhsT=wd64[:], rhs=P[:],
                                 start=True, stop=True)
                e64 = sb3.tile([64, 512], BF16, tag="e64")
                nc.scalar.activation(out=e64[:], in_=scp[:], func=Exp)
                # fused: em = e64*m64 and dsum = row-sum in one DVE op
                em = sb3.tile([64, 512], BF16, tag="em")
                dsum = sb3.tile([64, 1], F32, tag="dsum")
                nc.vector.scalar_tensor_tensor(
                    out=em[:], in0=e64[:], scalar=1.0, in1=m64[:],
                    op0=Mult, op1=Mult, accum_out=dsum[:])
                dinv = sb3.tile([64, 1], F32, tag="dinv")
                nc.vector.reciprocal(out=dinv[:], in_=dsum[:])
                # U = (wn * wdinv*dinv) * em  via 4x ts + 2x tt
                nrow = sb3.tile([64, 1], F32, tag="nrow")
                nc.vector.tensor_tensor(out=nrow[:], in0=wdinv[:],
                                        in1=dinv[:], op=Mult)
                wnn = sb3.tile([64, 512], BF16, tag="wnn")
                nc.vector.tensor_scalar(
                    out=wnn[:], in0=wn[:], scalar1=nrow[:], scalar2=None,
                    op0=Mult)
                U = sb3.tile([64, 512], BF16, tag="U")
                nc.vector.tensor_tensor(out=U[:], in0=wnn[:], in1=em[:],
                                        op=Mult)

                Ups = psu.tile([128, 4, 64], BF16, tag="Ups")
                for j2 in range(4):
                    nc.tensor.transpose(
                        out=Ups[:, j2, :],
                        in_=U[:, 128 * j2:128 * j2 + 128],
                        identity=ident[0:64, 0:64])
                Usb = sb3.tile([128, 4, 64], BF16, tag="Usb")
                nc.vector.tensor_copy(out=Usb[:], in_=Ups[:])
                for j2 in range(4):
                    s0 = 8 * st + 2 * j2
                    rhs = _ap(Usb[:, j2, :], 2 * j2,
                              [[Usb[:].ap[0][0], 128], [1, 2], [8, 8]])
                    for m2 in (0, 1):
                        nc.tensor.matmul(
                            out=yTp[:, m2, s0:s0 + 2, :],
                            lhsT=x[:, 4 * st + j2, 128 * m2:128 * m2 + 128],
                            rhs=rhs, start=True, stop=True)

            # ---- chunk tail: cpool, nv, output -----------------------------
            yTs = sb2.tile([128, 2, 32, 8], BF16, tag="yTs")
            nc.scalar.activation(out=yTs[:], in_=yTp[:], func=Copy)

            cpp = ps2.tile([128, 2, 32], F32, tag="tiny")
            for h in range(H):
                hc, hr = h // 4, h % 4
                for m2 in (0, 1):
                    # head h's column per seq: col = s*8 + h
                    rhs = _ap(yTs[:, m2, 0, 0:1], h,
                              [[yTs[:].ap[0][0], 128], [8, 32]])
                    nc.tensor.matmul(
                        out=cpp[32 * hr:32 * hr + 32, hc, :],
                        lhsT=wv[:, m2, 32 * h:32 * h + 32],
                        rhs=rhs, start=(m2 == 0), stop=(m2 == 1),
                        tile_position=(0, 32 * hr))
            cps = sb2.tile([128, 2, 32], BF16, tag="cps")
            nc.scalar.activation(out=cps[:], in_=cpp[:], func=Copy)

            nvp = ps2.tile([128, 2, 32], F32, tag="tiny")
            for do in (0, 1):
                for k2 in (0, 1):
                    nc.tensor.matmul(
                        out=nvp[:, do, :],
                        lhsT=wo[:, k2, 128 * do:128 * do + 128],
                        rhs=cps[:, k2, :],
                        start=(k2 == 0), stop=(k2 == 1))
            nvs = sb2.tile([128, 2, 32], BF16, tag="nvs")
            nc.scalar.activation(out=nvs[:], in_=nvp[:], func=Copy)

            nvt = ps2.tile([32, 2, 128], BF16, tag="tiny")
            for do in (0, 1):
                nc.tensor.transpose(out=nvt[:, do, :], in_=nvs[:, do, :],
                                    identity=ident[:])
            nvo = outp.tile([32, 256], F32, tag="nvo")
            nc.scalar.activation(out=nvo[:], in_=nvt[:].rearrange(
                "p a b -> p (a b)"), func=Copy)
            nc.sync.dma_start(
                out=out_d[SEQ_PER_CHUNK * c:SEQ_PER_CHUNK * (c + 1), :],
                in_=nvo[:])


# ---------------------------------------------------------------------------
def _host_prep(tokens, emb_table, WQ, WK, WV, WO, dense_w, dense_b,
               n_chunks=CHUNKS):
    """Build per-core input maps (numpy only)."""
    tokens = np.asarray(tokens)
    embf = np.ascontiguousarray(
        np.asarray(emb_table, dtype=np.float32).astype(NPBF))
    WQ = np.asarray(WQ, np.float32); WK = np.asarray(WK, np.float32)
    WV = np.asarray(WV, np.float32); WO = np.asarray(WO, np.float32)
    dwo = (WO @ np.asarray(dense_w, np.float32)[:, 0]).astype(np.float32)

    def pack(mat):  # [256, 256] -> [128, 2*256] (row 128k+p -> [p, k*256:])
        return np.ascontiguousarray(
            mat.reshape(2, 128, 256).transpose(1, 0, 2).reshape(128, 512)
        ).astype(NPBF)

    consts = {
        "wq": pack(WQ / L), "wk": pack(WK / L),
        "wkt": pack(np.ascontiguousarray(WK.T)),
        "wqt": pack(np.ascontiguousarray(WQ.T)),
        "wv": pack(WV), "wo": pack(WO),
    }
    # M8[:, h] = WV[:, hb] @ dwo[hb]; wpsc[p, k, 8h+s] = M8[128k+p, h]
    M8 = np.stack([WV[:, 32 * h:32 * h + 32] @ dwo[32 * h:32 * h + 32]
                   for h in range(H)], axis=1)  # [256, 8]
    wpsc = np.repeat(M8.reshape(2, 128, 1, 8), 8, axis=2)  # [2,128,8s,8h]
    # need col index 8h+s: wpsc[k,p, s, h] -> transpose to [p, k, h, s]? col=8h+s
    wpsc = wpsc.transpose(1, 0, 3, 2).reshape(128, 2 * 64)  # [p, k*(8h+s)]
    consts["wpsc"] = np.ascontiguousarray(wpsc).astype(NPBF)

    mh = np.zeros((128, 2, 256), np.float32)
    for p in range(128):
        for k in (0, 1):
            h_of_p = 4 * k + p // 32
            for st in range(4):
                for s in range(8):
                    mh[p, k, 64 * st + 8 * h_of_p + s] = 1.0
    consts["maskhead"] = mh.reshape(128, 512).astype(NPBF)
    consts["ident"] = np.eye(128, dtype=np.float32).astype(NPBF)
    m64 = np.zeros((64, 512), np.float32)
    for h in range(8):
        for s in range(8):
            m64[8 * h + s, 64 * s:64 * s + 64] = 1.0
    consts["mask64"] = m64.astype(NPBF)
    s64 = np.zeros((64, 64), np.float32)
    for h in range(8):
        for h2 in range(8):
            for s in range(8):
                s64[8 * h + s, 8 * h2 + s] = 1.0
    consts["seld64"] = s64.astype(NPBF)

    in_maps = []
    for core in range(NCORES):
        tc_ = tokens[SEQ_PER_CORE * core:SEQ_PER_CORE * (core + 1)]
        flat = np.asarray(tc_, np.int32).reshape(-1)
        idx = np.ascontiguousarray(
            flat[: n_chunks * TOK_PER_CHUNK].reshape(-1, 128).T)
        m = {"emb": embf, "idx": idx.astype(np.int32)}
        m.update(consts)
        in_maps.append(m)
    return in_maps


_NC_CACHE = {}


def kernel(tokens, emb_table, WQ, WK, WV, WO, dense_w, dense_b,
           n_chunks=CHUNKS, trace=False):
    if n_chunks not in _NC_CACHE:
        _NC_CACHE[n_chunks] = build_nc(n_chunks)
    nc = _NC_CACHE[n_chunks]
    in_maps = _host_prep(tokens, emb_table, WQ, WK, WV, WO, dense_w, dense_b,
                         n_chunks)
    res = run_bass_kernel_spmd(nc, in_maps, list(range(NCORES)), trace=trace)
    out = np.concatenate([r["out"] for r in res.results], axis=0)
    kernel._last_results = res
    return out


if __name__ == "__main__":
    # smoke test against numpy reference on small slice
    rng = np.random.default_rng(0)
    tokens = rng.integers(0, VOCAB, (B, L)).astype(np.int32)
    emb = (rng.standard_normal((VOCAB, D)) * 0.02).astype(np.float32)
    ws = [(rng.standard_normal((D, D)) * 0.02).astype(np.float32)
          for _ in range(4)]
    dw = (rng.standard_normal((D, 1)) * 0.02).astype(np.float32)
    db = np.zeros((1,), np.float32)
    out = kernel(tokens, emb, *ws, dw, db)
    print("out", out.shape, out.dtype, np.abs(out).mean())

